# revision 47
# baseline (speedup 1.0000x reference)
"""KGE scoring kernel for Trainium2 (8 NeuronCores, entity-table row-sharded).

score[b, n] = GAMMA - sum_d |h_n[b, d] - t_n[b, n, d]|
  h_n / t_n = L2-normalized Linear(concat(ent_emb[idx], rel_half))

The 200k x 256 entity table is row-sharded across the 8 cores, compacted
to referenced rows, and int8 per-row quantized (~5MB/core on the wire
instead of a 205MB f32 replica).  A tail pair (b, n) is computed on the
core owning row tail[b, n]; the host packs, per (core, b), the matching
n's into two 128-wide tiles (cap 256; Binomial(1024, 1/8) never exceeds
it, and a numpy fallback covers the impossible overflow) and scatters the
scalar scores back.  Head/relation rows and W are host-prepped (gather +
transpose + bf16); all FC compute, normalization and scoring stay on
device.

The main loop is a hardware For_i over batch rows (2 rows per iteration)
so the program stays ~150 instructions — full unrolling made every warm
call spend ~0.5s re-serializing and re-verifying a ~7k-instruction BIR.
Per-iteration operands (indices, scales, C_t/H_n rows) are staged with
small dynamically-sliced DMAs so compute APs stay static.

Per batch row b:
  t_fc = W1 @ t + C_t[b],  C_t = W2 @ re_t + b_fc  (per-b constant).
  After norm^2 (ACT Square+accum_out) and beta = ||t_fc||, a K=1 PE matmul
  accumulates -beta (x) h_n into the same PSUM, so
  score = GAMMA - (1/beta) * sum_d |psum|  (one DVE abs-add reduce per tile).
"""

import os
import sys

# Persistent XLA compilation cache: run_bass_via_pjrt builds a fresh jit
# closure per call, so without this every warm call re-runs the full
# client-side walrus/NEFF pipeline.  (No-op when the backend doesn't
# support serialized executables, but harmless.)
os.environ.setdefault("JAX_COMPILATION_CACHE_DIR", "/tmp/jax_comp_cache")
os.environ.setdefault("JAX_PERSISTENT_CACHE_MIN_COMPILE_TIME_SECS", "0")
os.environ.setdefault("JAX_PERSISTENT_CACHE_MIN_ENTRY_SIZE_BYTES", "0")

if "/opt/trn_rl_repo" not in sys.path:
    sys.path.insert(0, "/opt/trn_rl_repo")

import numpy as np
import ml_dtypes

import concourse.bacc as bacc
import concourse.mybir as mybir
import concourse.tile as tile
from concourse.bass import IndirectOffsetOnAxis, ds
from concourse.bass_utils import run_bass_kernel_spmd
from concourse.masks import make_identity

GAMMA = 12.0
NENTITY = 200000
NREL = 500
D = 256          # hidden
B_FULL = 256     # total batch
NEG = 1024
NCORES = 8
SHARD = NENTITY // NCORES   # 25000 entity rows per core
# compacted shard capacity: only rows actually referenced are shipped
# (expected ~18.3k of 25k, observed max 18411; exact host fallback beyond)
CROWS = 18432
CAP = 256        # max pairs per (core, b); NTILE tiles of 128
NTILE = CAP // 128
NCOLS = B_FULL * NTILE      # 512 score columns per core
UNROLL = 2       # batch rows per hardware-loop iteration
BF16 = mybir.dt.bfloat16
F32 = mybir.dt.float32
I32 = mybir.dt.int32
I8 = mybir.dt.int8
U16 = mybir.dt.uint16
F16 = mybir.dt.float16
Square = mybir.ActivationFunctionType.Square
Alu = mybir.AluOpType
BFNP = ml_dtypes.bfloat16


def build_kernel(nc):
    """Emit the SPMD per-core program."""
    # Inputs are merged into few tensors — the axon tunnel charges ~11ms
    # per input array on top of ~13ms/MB, so fewer/larger arrays win.
    # int8 per-row symmetric quantized compacted shard; cols D:D+2 of each
    # row carry its f16 dequant scale (bit-packed), so one gather brings
    # row + scale together
    ent = nc.dram_tensor("ent", [CROWS, D + 2], I8, kind="ExternalInput").ap()
    # int8 operands, pre-transposed on host: W chunks | head rows | rel rows
    #   dequant(iops[:, 256j:...])[p, d]       = W_fc[d, 128*j + p]        j<4
    #   dequant(iops[:, 1024+256j:...])[p, b]  = ent_emb[head[b], 128j+p]  j<2
    #   dequant(iops[:, 1536+256j:...])[p, b]  = rel_emb[relation[b], 128j+p]
    iops = nc.dram_tensor("iops", [128, 10 * D], I8, kind="ExternalInput").ap()
    # per-row dequant scales for iops: w[0:4] | h[4:6] | r[6:10];
    # cols 10:12 carry the f32 bias as [128, 2] (relayout to [1, 256] on
    # device via a DRAM round trip)
    scls = nc.dram_tensor("scls", [128, 12], F32, kind="ExternalInput").ap()
    # packed local tail rows (col r = b*NTILE + g, row p -> slot g*128 + p)
    idxscl = nc.dram_tensor("idxscl", [128, NCOLS], U16,
                            kind="ExternalInput").ap()
    # scores in [p, col] layout; host transposes
    out = nc.dram_tensor("out", [128, NCOLS], F16, kind="ExternalOutput").ap()

    with tile.TileContext(nc) as tc:
        with (
            tc.tile_pool(name="const", bufs=1) as cpool,
            tc.tile_pool(name="stage", bufs=2) as spool,
            tc.tile_pool(name="gath", bufs=2) as gpool,
            tc.tile_pool(name="tt", bufs=2) as ttpool,
            tc.tile_pool(name="work", bufs=2) as wpool,
            tc.tile_pool(name="dram", bufs=1, space="DRAM") as dpool,
            tc.tile_pool(name="pstt", bufs=2, space="PSUM") as ps_tt,
            tc.tile_pool(name="psbt", bufs=1, space="PSUM") as ps_bt,
            tc.tile_pool(name="psmain", bufs=2, space="PSUM") as psmain,
        ):
            # ---- constants ----
            ident = cpool.tile([128, 128], BF16)
            make_identity(nc, ident[:])
            ones_row = cpool.tile([1, 128], BF16)
            nc.vector.memset(ones_row[:], 1.0)

            # load int8 operands + dequant to bf16 (per-partition-row scales)
            iq = cpool.tile([128, 10, D], I8, tag="iq")
            nc.sync.dma_start(iq[:], iops[:, :])
            scl_sb = cpool.tile([128, 12], F32, tag="scl")
            nc.sync.dma_start(scl_sb[:], scls[:, :])
            wt = cpool.tile([128, 4, D], BF16, tag="wt")
            for j in range(4):
                nc.vector.tensor_scalar_mul(wt[:, j, :], iq[:, j, :],
                                            scl_sb[:, j:j + 1])
            ht = cpool.tile([128, 2, B_FULL], BF16, tag="ht")
            for j in range(2):
                nc.vector.tensor_scalar_mul(ht[:, j, :], iq[:, 4 + j, :],
                                            scl_sb[:, 4 + j:5 + j])
            rt = cpool.tile([128, 4, B_FULL], BF16, tag="rt")
            for j in range(4):
                nc.vector.tensor_scalar_mul(rt[:, j, :], iq[:, 6 + j, :],
                                            scl_sb[:, 6 + j:7 + j])
            # bias rides in scls cols 10:12; relayout [128, 2] -> [1, 256]
            bd = dpool.tile([128, 2], F32, tag="bd")
            nc.sync.dma_start(bd[:], scl_sb[:, 10:12])
            b_f32 = cpool.tile([1, D], F32, tag="bf32")
            nc.sync.dma_start(b_f32[:], bd[:, :])
            b_bf = cpool.tile([1, D], BF16, tag="bias")
            nc.vector.tensor_copy(b_bf[:], b_f32[:])

            # ---- per-b constants for ALL 256 b, in two chunks of 128 ----
            # C_t[b,:] = W2 @ re_t[b] + b_fc ; Hn[b,:] = normalize(FC(head))
            ctd = dpool.tile([B_FULL, D], BF16, tag="ctd")
            hnd = dpool.tile([B_FULL, D], BF16, tag="hnd")
            for ch in range(2):
                bs = 128 * ch
                ct_ps = ps_tt.tile([128, D], F32, tag="ttp")
                nc.tensor.matmul(ct_ps[:], lhsT=ones_row[:], rhs=b_bf[:],
                                 start=True, stop=False)
                nc.tensor.matmul(ct_ps[:], lhsT=rt[:, 2, bs:bs + 128],
                                 rhs=wt[:, 2, :], start=False, stop=False)
                nc.tensor.matmul(ct_ps[:], lhsT=rt[:, 3, bs:bs + 128],
                                 rhs=wt[:, 3, :], start=False, stop=True)
                ct = wpool.tile([128, D], BF16, tag="ct")
                nc.scalar.copy(ct[:], ct_ps[:])
                nc.sync.dma_start(ctd[bs:bs + 128, :], ct[:])

                hf_ps = ps_tt.tile([128, D], F32, tag="ttp")
                nc.tensor.matmul(hf_ps[:], lhsT=ones_row[:], rhs=b_bf[:],
                                 start=True, stop=False)
                nc.tensor.matmul(hf_ps[:], lhsT=ht[:, 0, bs:bs + 128],
                                 rhs=wt[:, 0, :], start=False, stop=False)
                nc.tensor.matmul(hf_ps[:], lhsT=ht[:, 1, bs:bs + 128],
                                 rhs=wt[:, 1, :], start=False, stop=False)
                nc.tensor.matmul(hf_ps[:], lhsT=rt[:, 0, bs:bs + 128],
                                 rhs=wt[:, 2, :], start=False, stop=False)
                nc.tensor.matmul(hf_ps[:], lhsT=rt[:, 1, bs:bs + 128],
                                 rhs=wt[:, 3, :], start=False, stop=True)
                h_sq = wpool.tile([128, D], BF16, tag="hsq")
                h_nn = wpool.tile([128, 1], F32, tag="hnn")
                nc.scalar.activation(h_sq[:], hf_ps[:], Square, accum_out=h_nn[:])
                h_beta = wpool.tile([128, 1], F32, tag="hbeta")
                nc.scalar.sqrt(h_beta[:], h_nn[:])
                h_rs = wpool.tile([128, 1], F32, tag="hrs")
                nc.vector.reciprocal_approx_fast(h_rs[:], h_beta[:])
                hn = wpool.tile([128, D], BF16, tag="hn")
                nc.vector.tensor_scalar_mul(hn[:], hf_ps[:], h_rs[:, :1])
                nc.sync.dma_start(hnd[bs:bs + 128, :], hn[:])

            # ---- main hardware loop: UNROLL batch rows per iteration ----
            NC_IT = NTILE * UNROLL      # score cols per iteration
            with tc.For_i(0, B_FULL // UNROLL, 1) as it:
                # stage this iteration's operands (dynamic DRAM slices)
                tix = spool.tile([128, NC_IT], U16, tag="tix")
                nc.sync.dma_start(tix[:], idxscl[:, ds(it * NC_IT, NC_IT)])
                tii = spool.tile([128, NC_IT], I32, tag="tii")
                nc.vector.tensor_copy(tii[:], tix[:])
                crow = spool.tile([1, UNROLL, D], BF16, tag="crow")
                nc.sync.dma_start(crow[:], ctd[ds(it * UNROLL, UNROLL), :])
                hrow = spool.tile([1, UNROLL, D], BF16, tag="hrow")
                nc.sync.dma_start(hrow[:], hnd[ds(it * UNROLL, UNROLL), :])
                scu = spool.tile([128, NC_IT], F16, tag="scu")

                for u in range(UNROLL):
                    # gather 2x128 packed tail rows (one DMA per 128-row
                    # tile: single-column offset APs only — multi-column
                    # offsets misbehave on HW SWDGE)
                    gq = gpool.tile([128, NTILE, D + 2], I8, tag="gq")
                    for g in range(NTILE):
                        k = NTILE * u + g
                        nc.gpsimd.indirect_dma_start(
                            out=gq[:, g, :], out_offset=None, in_=ent[:],
                            in_offset=IndirectOffsetOnAxis(
                                ap=tii[:, k:k + 1], axis=0))
                    # dequant: t = q * s_row; the row's f16 scale rides in
                    # its last 2 bytes (widen to f32 for the scalar AP)
                    gsc = wpool.tile([128, NTILE], F32, tag="gsc")
                    for g in range(NTILE):
                        nc.vector.tensor_copy(
                            gsc[:, g:g + 1], gq[:, g, D:D + 2].bitcast(F16))
                    gt = gpool.tile([128, NTILE, D], BF16, tag="gt")
                    for g in range(NTILE):
                        nc.vector.tensor_scalar_mul(
                            gt[:, g, :], gq[:, g, 0:D], gsc[:, g:g + 1])
                    nn2 = wpool.tile([128, NTILE], F32, tag="nn2")
                    ps_tiles = [psmain.tile([128, D], F32, tag="psm",
                                            name=f"psm_{u}_{i}")[:]
                                for i in range(NTILE)]
                    for g in range(NTILE):
                        # transpose tile -> TT [128, 2, 128] (k-chunk, rows)
                        ttp = ps_tt.tile([128, 2, 128], BF16, tag="ttp")
                        nc.tensor.transpose(ttp[:, 0, :], gt[:, g, 0:128],
                                            ident[:])
                        nc.tensor.transpose(ttp[:, 1, :], gt[:, g, 128:256],
                                            ident[:])
                        tt = ttpool.tile([128, 2, 128], BF16, tag="tt")
                        nc.scalar.copy(tt[:, 0, :], ttp[:, 0, :])
                        nc.vector.tensor_copy(tt[:, 1, :], ttp[:, 1, :])
                        # psum = C_t[b] + W1 @ t
                        ps = ps_tiles[g]
                        nc.tensor.matmul(ps, lhsT=ones_row[:],
                                         rhs=crow[0:1, u, :],
                                         start=True, stop=False)
                        nc.tensor.matmul(ps, lhsT=tt[:, 0, :],
                                         rhs=wt[:, 0, :],
                                         start=False, stop=False)
                        nc.tensor.matmul(ps, lhsT=tt[:, 1, :],
                                         rhs=wt[:, 1, :],
                                         start=False, stop=True)
                        # norm^2 -> nn2 col g
                        sq = wpool.tile([128, D], BF16, tag="sq")
                        nc.scalar.activation(sq[:], ps, Square,
                                             accum_out=nn2[:, g:g + 1])
                    # beta = sqrt(nn); negated row form for the K=1 correction
                    beta = wpool.tile([128, NTILE], F32, tag="beta")
                    nc.scalar.sqrt(beta[:], nn2[:])
                    nbeta = wpool.tile([128, NTILE], BF16, tag="nbeta")
                    nc.vector.tensor_scalar_mul(nbeta[:], beta[:], -1.0)
                    rs = wpool.tile([128, NTILE], F32, tag="rs")
                    nc.vector.reciprocal_approx_fast(rs[:], beta[:])
                    nrs = wpool.tile([128, NTILE], F32, tag="nrs")
                    nc.vector.tensor_scalar_mul(nrs[:], rs[:], -1.0)
                    btp = ps_bt.tile([1, NTILE, 128], BF16, tag="btp")
                    for g in range(NTILE):
                        nc.tensor.transpose(btp[0:1, g, :],
                                            nbeta[:, g:g + 1], ident[:])
                    bt = wpool.tile([1, NTILE, 128], BF16, tag="bt")
                    nc.vector.tensor_copy(bt[:], btp[:])
                    for g in range(NTILE):
                        k = NTILE * u + g
                        ps = ps_tiles[g]
                        # psum -= beta (x) h_n
                        nc.tensor.matmul(ps, lhsT=bt[0:1, g, :],
                                         rhs=hrow[0:1, u, :],
                                         start=False, stop=True,
                                         skip_group_check=True)
                        scol = wpool.tile([128, 1], F32, tag="scol")
                        nc.vector.tensor_reduce(
                            scol[:], ps, mybir.AxisListType.X, Alu.add,
                            apply_absolute_value=True)
                        # score = GAMMA - s/beta = s * (-rs) + GAMMA
                        nc.vector.tensor_scalar(
                            out=scu[:, k:k + 1],
                            in0=scol[:], scalar1=nrs[:, g:g + 1],
                            scalar2=GAMMA, op0=Alu.mult, op1=Alu.add)
                nc.sync.dma_start(out[:, ds(it * NC_IT, NC_IT)], scu[:])

    return nc


def make_in_maps(head, tail, relation, entity_emb, relation_emb, W_fc, b_fc):
    """Shard/pack FULL inputs into 8 per-core input maps.

    Returns (in_maps, nmaps, overflow) where nmaps[c][b, slot] = n (or -1
    for padding) maps device scores back, and overflow lists (b, n) pairs
    that did not fit (computed on host; probability ~0).
    """
    head = np.asarray(head).astype(np.int64).reshape(B_FULL)
    tail = np.asarray(tail).astype(np.int64)
    relation = np.asarray(relation).astype(np.int64).reshape(B_FULL)
    entity_emb = np.asarray(entity_emb, dtype=np.float32)
    relation_emb = np.asarray(relation_emb, dtype=np.float32)
    W_fc = np.asarray(W_fc, dtype=np.float32)
    b_fc = np.asarray(b_fc, dtype=np.float32).reshape(1, D)

    def quant_rows(x, nchunk, width):
        """Per-(partition, chunk) row int8 quant of a [128, nchunk*width]
        layout; returns (int8 data, [128, nchunk] f32 scales)."""
        r = x.reshape(128, nchunk, width)
        s = np.maximum(np.abs(r).max(axis=2), 1e-12) / 127.0   # [128, nchunk]
        q = np.round(r / s[:, :, None]).astype(np.int8)
        return (np.ascontiguousarray(q.reshape(128, nchunk * width)),
                np.ascontiguousarray(s.astype(np.float32)))

    # shared host-prepped operands (int8 + scales, matmul-ready layouts),
    # merged into one i8 tensor + one f32 scale tensor
    wt_f = np.ascontiguousarray(
        W_fc.T.reshape(4, 128, D).transpose(1, 0, 2).reshape(128, 4 * D))
    wt, wscl = quant_rows(wt_f, 4, D)
    hrows = entity_emb[head]                       # [B, D]
    ht_f = np.ascontiguousarray(
        hrows.T.reshape(2, 128, B_FULL).transpose(1, 0, 2).reshape(128, 2 * B_FULL))
    ht, hscl = quant_rows(ht_f, 2, B_FULL)
    rrows = relation_emb[relation]                 # [B, 2D]
    rt_f = np.ascontiguousarray(
        rrows.T.reshape(4, 128, B_FULL).transpose(1, 0, 2).reshape(128, 4 * B_FULL))
    rt, rscl = quant_rows(rt_f, 4, B_FULL)
    iops = np.ascontiguousarray(np.concatenate([wt, ht, rt], axis=1))
    scls = np.ascontiguousarray(np.concatenate(
        [wscl, hscl, rscl, b_fc.reshape(128, 2)], axis=1).astype(np.float32))
    # int8 per-row symmetric quantization of the entity table
    scl = np.maximum(np.abs(entity_emb).max(axis=1) / 127.0, 1e-12)  # [N]
    ent_q = np.round(entity_emb / scl[:, None]).astype(np.int8)

    owner = tail // SHARD
    local = (tail % SHARD).astype(np.int32)
    in_maps, nmaps, overflow = [], [], []
    for c in range(NCORES):
        mask = owner == c
        # compact the shard to referenced rows only (remapped indices)
        rows = np.unique(local[mask])
        if len(rows) > CROWS:
            # drop pairs referencing the rarest overflow rows -> host path
            drop_rows = set(rows[CROWS:].tolist())
            over_r = mask & np.isin(local, list(drop_rows))
            ob, on = np.nonzero(over_r)
            overflow.extend(zip(ob.tolist(), on.tolist()))
            mask = mask & ~over_r
            rows = rows[:CROWS]
        remap = np.zeros(SHARD, np.int32)
        remap[rows] = np.arange(len(rows), dtype=np.int32)
        ent_c = np.zeros((CROWS, D + 2), np.int8)
        ent_c[:len(rows), :D] = ent_q[c * SHARD + rows]
        ent_c[:len(rows), D:] = (
            scl[c * SHARD + rows].astype(np.float16).view(np.int8).reshape(-1, 2))

        pos = np.cumsum(mask, axis=1) - 1
        over = mask & (pos >= CAP)
        if over.any():
            ob, on = np.nonzero(over)
            overflow.extend(zip(ob.tolist(), on.tolist()))
            mask = mask & ~over
        idx_c = np.zeros((B_FULL, CAP), np.int32)
        nmap = np.full((B_FULL, CAP), -1, np.int64)
        bb, nn_ = np.nonzero(mask)
        p = pos[mask]
        idx_c[bb, p] = local[mask]
        nmap[bb, p] = nn_
        idx_cr = remap[idx_c]
        tidx_c = idx_cr.reshape(NCOLS, 128).T.astype(np.uint16)
        in_maps.append({
            "ent": ent_c,
            "iops": iops,
            "scls": scls,
            "idxscl": np.ascontiguousarray(tidx_c),
        })
        nmaps.append(nmap)
    return in_maps, nmaps, overflow


def _host_scores(head, tail, relation, entity_emb, relation_emb, W_fc, b_fc,
                 pairs):
    """Exact numpy scores for a small list of (b, n) pairs (overflow path)."""
    b_idx = np.array([p[0] for p in pairs])
    n_idx = np.array([p[1] for p in pairs])
    h = entity_emb[head.reshape(-1)[b_idx]]
    t = entity_emb[tail[b_idx, n_idx]]
    r = relation_emb[relation.reshape(-1)[b_idx]]
    re_h, re_t = r[:, :D], r[:, D:]
    hc = np.concatenate([h, re_h], -1)
    tc = np.concatenate([t, re_t], -1)
    hf = hc @ W_fc.T + b_fc.reshape(-1)
    tf = tc @ W_fc.T + b_fc.reshape(-1)
    hn = hf / np.maximum(np.linalg.norm(hf, axis=-1, keepdims=True), 1e-12)
    tn = tf / np.maximum(np.linalg.norm(tf, axis=-1, keepdims=True), 1e-12)
    return b_idx, n_idx, GAMMA - np.abs(hn - tn).sum(-1)


def unshard_output(res, nmaps):
    """Scatter per-core packed scores back to the FULL [B, NEG] output."""
    score = np.zeros((B_FULL, NEG), dtype=np.float32)
    for c in range(NCORES):
        # out is [p, col]; col = b*NTILE + g, slot = g*128 + p
        s_c = np.ascontiguousarray(
            res[c]["out"].astype(np.float32).T).reshape(B_FULL, CAP)
        nmap = nmaps[c]
        vb, vs = np.nonzero(nmap >= 0)
        score[vb, nmap[vb, vs]] = s_c[vb, vs]
    return score


def kernel(head, tail, relation, entity_emb, relation_emb, W_fc, b_fc):
    nc = bacc.Bacc("TRN2", target_bir_lowering=False, debug=False)
    build_kernel(nc)
    nc.compile()
    in_maps, nmaps, overflow = make_in_maps(
        head, tail, relation, entity_emb, relation_emb, W_fc, b_fc)
    res = run_bass_kernel_spmd(nc, in_maps, core_ids=list(range(NCORES)))
    score = unshard_output(res.results, nmaps)
    if overflow:
        bi, ni, sv = _host_scores(
            np.asarray(head), np.asarray(tail), np.asarray(relation),
            np.asarray(entity_emb, np.float32),
            np.asarray(relation_emb, np.float32),
            np.asarray(W_fc, np.float32), np.asarray(b_fc, np.float32),
            overflow)
        score[bi, ni] = sv
    return score


# revision 52
# speedup vs baseline: 1.2003x; 1.2003x over previous
"""KGE scoring kernel for Trainium2 (8 NeuronCores, entity-table row-sharded).

score[b, n] = GAMMA - sum_d |h_n[b, d] - t_n[b, n, d]|
  h_n / t_n = L2-normalized Linear(concat(ent_emb[idx], rel_half))

The 200k x 256 entity table is row-sharded across the 8 cores, compacted
to referenced rows, and int8 per-row quantized (~5MB/core on the wire
instead of a 205MB f32 replica).  A tail pair (b, n) is computed on the
core owning row tail[b, n]; the host packs, per (core, b), the matching
n's into two 128-wide tiles (cap 256; Binomial(1024, 1/8) never exceeds
it, and a numpy fallback covers the impossible overflow) and scatters the
scalar scores back.  Head/relation rows and W are host-prepped (gather +
transpose + bf16); all FC compute, normalization and scoring stay on
device.

The main loop is a hardware For_i over batch rows (2 rows per iteration)
so the program stays ~150 instructions — full unrolling made every warm
call spend ~0.5s re-serializing and re-verifying a ~7k-instruction BIR.
Per-iteration operands (indices, scales, C_t/H_n rows) are staged with
small dynamically-sliced DMAs so compute APs stay static.

Per batch row b:
  t_fc = W1 @ t + C_t[b],  C_t = W2 @ re_t + b_fc  (per-b constant).
  After norm^2 (ACT Square+accum_out) and beta = ||t_fc||, a K=1 PE matmul
  accumulates -beta (x) h_n into the same PSUM, so
  score = GAMMA - (1/beta) * sum_d |psum|  (one DVE abs-add reduce per tile).
"""

import os
import sys

# Persistent XLA compilation cache: run_bass_via_pjrt builds a fresh jit
# closure per call, so without this every warm call re-runs the full
# client-side walrus/NEFF pipeline.  (No-op when the backend doesn't
# support serialized executables, but harmless.)
os.environ.setdefault("JAX_COMPILATION_CACHE_DIR", "/tmp/jax_comp_cache")
os.environ.setdefault("JAX_PERSISTENT_CACHE_MIN_COMPILE_TIME_SECS", "0")
os.environ.setdefault("JAX_PERSISTENT_CACHE_MIN_ENTRY_SIZE_BYTES", "0")

if "/opt/trn_rl_repo" not in sys.path:
    sys.path.insert(0, "/opt/trn_rl_repo")

import numpy as np
import ml_dtypes

import concourse.bacc as bacc
import concourse.mybir as mybir
import concourse.tile as tile
from concourse.bass import IndirectOffsetOnAxis, ds
from concourse.bass_utils import run_bass_kernel_spmd
from concourse.masks import make_identity

GAMMA = 12.0
NENTITY = 200000
NREL = 500
D = 256          # hidden
B_FULL = 256     # total batch
NEG = 1024
NCORES = 8
SHARD = NENTITY // NCORES   # 25000 entity rows per core
# compacted shard capacity: only rows actually referenced are shipped
# (expected ~18.3k of 25k, observed max 18411; exact host fallback beyond)
CROWS = 18432
CAP = 256        # max pairs per (core, b); NTILE tiles of 128
NTILE = CAP // 128
NCOLS = B_FULL * NTILE      # 512 score columns per core
UNROLL = 2       # batch rows per hardware-loop iteration
BF16 = mybir.dt.bfloat16
F32 = mybir.dt.float32
I32 = mybir.dt.int32
I8 = mybir.dt.int8
U16 = mybir.dt.uint16
F16 = mybir.dt.float16
Square = mybir.ActivationFunctionType.Square
Alu = mybir.AluOpType
BFNP = ml_dtypes.bfloat16


def build_kernel(nc):
    """Emit the SPMD per-core program."""
    # Inputs are merged into few tensors — the axon tunnel charges ~11ms
    # per input array on top of ~13ms/MB, so fewer/larger arrays win.
    # int8 per-row symmetric quantized compacted shard; cols D:D+2 of each
    # row carry its f16 dequant scale (bit-packed), so one gather brings
    # row + scale together
    ent = nc.dram_tensor("ent", [CROWS, D + 2], I8, kind="ExternalInput").ap()
    # everything else rides in ONE byte blob (bitcast views per region):
    #   bytes [0, 2560): int8 operands, pre-transposed on host:
    #     dequant([:, 256j:...])[p, d]       = W_fc[d, 128*j + p]        j<4
    #     dequant([:, 1024+256j:...])[p, b]  = ent_emb[head[b], 128j+p]  j<2
    #     dequant([:, 1536+256j:...])[p, b]  = rel_emb[relation[b], 128j+p]
    #   bytes [2560, 2608): f32 dequant scales w[0:4] | h[4:6] | r[6:10]
    #     and the f32 bias as [128, 2] in scale cols 10:12 (relayout to
    #     [1, 256] on device via a DRAM round trip)
    #   bytes [2608, 3632): u16 packed local tail rows
    #     (col r = b*NTILE + g, row p -> slot g*128 + p)
    IOPS_B, SCLS_B, TIDX_B = 0, 10 * D, 10 * D + 48
    blob = nc.dram_tensor("blob", [128, TIDX_B + 2 * NCOLS], I8,
                          kind="ExternalInput").ap()
    # scores in [p, col] layout; host transposes
    out = nc.dram_tensor("out", [128, NCOLS], F16, kind="ExternalOutput").ap()

    with tile.TileContext(nc) as tc:
        with (
            tc.tile_pool(name="const", bufs=1) as cpool,
            tc.tile_pool(name="stage", bufs=2) as spool,
            tc.tile_pool(name="gath", bufs=2) as gpool,
            tc.tile_pool(name="tt", bufs=2) as ttpool,
            tc.tile_pool(name="work", bufs=2) as wpool,
            tc.tile_pool(name="dram", bufs=1, space="DRAM") as dpool,
            tc.tile_pool(name="pstt", bufs=2, space="PSUM") as ps_tt,
            tc.tile_pool(name="psbt", bufs=1, space="PSUM") as ps_bt,
            tc.tile_pool(name="psmain", bufs=2, space="PSUM") as psmain,
        ):
            # ---- constants ----
            ident = cpool.tile([128, 128], BF16)
            make_identity(nc, ident[:])
            ones_row = cpool.tile([1, 128], BF16)
            nc.vector.memset(ones_row[:], 1.0)

            # load int8 operands + dequant to bf16 (per-partition-row scales)
            iq = cpool.tile([128, 10, D], I8, tag="iq")
            nc.sync.dma_start(iq[:], blob[:, IOPS_B:IOPS_B + 10 * D])
            scl_sb = cpool.tile([128, 12], F32, tag="scl")
            nc.sync.dma_start(scl_sb[:],
                              blob[:, SCLS_B:SCLS_B + 48].bitcast(F32))
            wt = cpool.tile([128, 4, D], BF16, tag="wt")
            for j in range(4):
                nc.vector.tensor_scalar_mul(wt[:, j, :], iq[:, j, :],
                                            scl_sb[:, j:j + 1])
            ht = cpool.tile([128, 2, B_FULL], BF16, tag="ht")
            for j in range(2):
                nc.vector.tensor_scalar_mul(ht[:, j, :], iq[:, 4 + j, :],
                                            scl_sb[:, 4 + j:5 + j])
            rt = cpool.tile([128, 4, B_FULL], BF16, tag="rt")
            for j in range(4):
                nc.vector.tensor_scalar_mul(rt[:, j, :], iq[:, 6 + j, :],
                                            scl_sb[:, 6 + j:7 + j])
            # bias rides in scls cols 10:12; relayout [128, 2] -> [1, 256]
            bd = dpool.tile([128, 2], F32, tag="bd")
            nc.sync.dma_start(bd[:], scl_sb[:, 10:12])
            b_f32 = cpool.tile([1, D], F32, tag="bf32")
            nc.sync.dma_start(b_f32[:], bd[:, :])
            b_bf = cpool.tile([1, D], BF16, tag="bias")
            nc.vector.tensor_copy(b_bf[:], b_f32[:])

            # ---- per-b constants for ALL 256 b, in two chunks of 128 ----
            # C_t[b,:] = W2 @ re_t[b] + b_fc ; Hn[b,:] = normalize(FC(head))
            ctd = dpool.tile([B_FULL, D], BF16, tag="ctd")
            hnd = dpool.tile([B_FULL, D], BF16, tag="hnd")
            for ch in range(2):
                bs = 128 * ch
                ct_ps = ps_tt.tile([128, D], F32, tag="ttp")
                nc.tensor.matmul(ct_ps[:], lhsT=ones_row[:], rhs=b_bf[:],
                                 start=True, stop=False)
                nc.tensor.matmul(ct_ps[:], lhsT=rt[:, 2, bs:bs + 128],
                                 rhs=wt[:, 2, :], start=False, stop=False)
                nc.tensor.matmul(ct_ps[:], lhsT=rt[:, 3, bs:bs + 128],
                                 rhs=wt[:, 3, :], start=False, stop=True)
                ct = wpool.tile([128, D], BF16, tag="ct")
                nc.scalar.copy(ct[:], ct_ps[:])
                nc.sync.dma_start(ctd[bs:bs + 128, :], ct[:])

                hf_ps = ps_tt.tile([128, D], F32, tag="ttp")
                nc.tensor.matmul(hf_ps[:], lhsT=ones_row[:], rhs=b_bf[:],
                                 start=True, stop=False)
                nc.tensor.matmul(hf_ps[:], lhsT=ht[:, 0, bs:bs + 128],
                                 rhs=wt[:, 0, :], start=False, stop=False)
                nc.tensor.matmul(hf_ps[:], lhsT=ht[:, 1, bs:bs + 128],
                                 rhs=wt[:, 1, :], start=False, stop=False)
                nc.tensor.matmul(hf_ps[:], lhsT=rt[:, 0, bs:bs + 128],
                                 rhs=wt[:, 2, :], start=False, stop=False)
                nc.tensor.matmul(hf_ps[:], lhsT=rt[:, 1, bs:bs + 128],
                                 rhs=wt[:, 3, :], start=False, stop=True)
                h_sq = wpool.tile([128, D], BF16, tag="hsq")
                h_nn = wpool.tile([128, 1], F32, tag="hnn")
                nc.scalar.activation(h_sq[:], hf_ps[:], Square, accum_out=h_nn[:])
                h_beta = wpool.tile([128, 1], F32, tag="hbeta")
                nc.scalar.sqrt(h_beta[:], h_nn[:])
                h_rs = wpool.tile([128, 1], F32, tag="hrs")
                nc.vector.reciprocal_approx_fast(h_rs[:], h_beta[:])
                hn = wpool.tile([128, D], BF16, tag="hn")
                nc.vector.tensor_scalar_mul(hn[:], hf_ps[:], h_rs[:, :1])
                nc.sync.dma_start(hnd[bs:bs + 128, :], hn[:])

            # ---- main hardware loop: UNROLL batch rows per iteration ----
            NC_IT = NTILE * UNROLL      # score cols per iteration
            with tc.For_i(0, B_FULL // UNROLL, 1) as it:
                # stage this iteration's operands (dynamic DRAM slices)
                tix = spool.tile([128, NC_IT], U16, tag="tix")
                nc.sync.dma_start(
                    tix[:],
                    blob[:, ds(TIDX_B + it * 2 * NC_IT, 2 * NC_IT)].bitcast(U16))
                tii = spool.tile([128, NC_IT], I32, tag="tii")
                nc.vector.tensor_copy(tii[:], tix[:])
                crow = spool.tile([1, UNROLL, D], BF16, tag="crow")
                nc.sync.dma_start(crow[:], ctd[ds(it * UNROLL, UNROLL), :])
                hrow = spool.tile([1, UNROLL, D], BF16, tag="hrow")
                nc.sync.dma_start(hrow[:], hnd[ds(it * UNROLL, UNROLL), :])
                scu = spool.tile([128, NC_IT], F16, tag="scu")

                for u in range(UNROLL):
                    # gather 2x128 packed tail rows (one DMA per 128-row
                    # tile: single-column offset APs only — multi-column
                    # offsets misbehave on HW SWDGE)
                    gq = gpool.tile([128, NTILE, D + 2], I8, tag="gq")
                    for g in range(NTILE):
                        k = NTILE * u + g
                        nc.gpsimd.indirect_dma_start(
                            out=gq[:, g, :], out_offset=None, in_=ent[:],
                            in_offset=IndirectOffsetOnAxis(
                                ap=tii[:, k:k + 1], axis=0))
                    # dequant: t = q * s_row; the row's f16 scale rides in
                    # its last 2 bytes (widen to f32 for the scalar AP)
                    gsc = wpool.tile([128, NTILE], F32, tag="gsc")
                    for g in range(NTILE):
                        nc.vector.tensor_copy(
                            gsc[:, g:g + 1], gq[:, g, D:D + 2].bitcast(F16))
                    gt = gpool.tile([128, NTILE, D], BF16, tag="gt")
                    for g in range(NTILE):
                        nc.vector.tensor_scalar_mul(
                            gt[:, g, :], gq[:, g, 0:D], gsc[:, g:g + 1])
                    nn2 = wpool.tile([128, NTILE], F32, tag="nn2")
                    ps_tiles = [psmain.tile([128, D], F32, tag="psm",
                                            name=f"psm_{u}_{i}")[:]
                                for i in range(NTILE)]
                    for g in range(NTILE):
                        # transpose tile -> TT [128, 2, 128] (k-chunk, rows)
                        ttp = ps_tt.tile([128, 2, 128], BF16, tag="ttp")
                        nc.tensor.transpose(ttp[:, 0, :], gt[:, g, 0:128],
                                            ident[:])
                        nc.tensor.transpose(ttp[:, 1, :], gt[:, g, 128:256],
                                            ident[:])
                        tt = ttpool.tile([128, 2, 128], BF16, tag="tt")
                        nc.scalar.copy(tt[:, 0, :], ttp[:, 0, :])
                        nc.vector.tensor_copy(tt[:, 1, :], ttp[:, 1, :])
                        # psum = C_t[b] + W1 @ t
                        ps = ps_tiles[g]
                        nc.tensor.matmul(ps, lhsT=ones_row[:],
                                         rhs=crow[0:1, u, :],
                                         start=True, stop=False)
                        nc.tensor.matmul(ps, lhsT=tt[:, 0, :],
                                         rhs=wt[:, 0, :],
                                         start=False, stop=False)
                        nc.tensor.matmul(ps, lhsT=tt[:, 1, :],
                                         rhs=wt[:, 1, :],
                                         start=False, stop=True)
                        # norm^2 -> nn2 col g
                        sq = wpool.tile([128, D], BF16, tag="sq")
                        nc.scalar.activation(sq[:], ps, Square,
                                             accum_out=nn2[:, g:g + 1])
                    # beta = sqrt(nn); negated row form for the K=1 correction
                    beta = wpool.tile([128, NTILE], F32, tag="beta")
                    nc.scalar.sqrt(beta[:], nn2[:])
                    nbeta = wpool.tile([128, NTILE], BF16, tag="nbeta")
                    nc.vector.tensor_scalar_mul(nbeta[:], beta[:], -1.0)
                    rs = wpool.tile([128, NTILE], F32, tag="rs")
                    nc.vector.reciprocal_approx_fast(rs[:], beta[:])
                    nrs = wpool.tile([128, NTILE], F32, tag="nrs")
                    nc.vector.tensor_scalar_mul(nrs[:], rs[:], -1.0)
                    btp = ps_bt.tile([1, NTILE, 128], BF16, tag="btp")
                    for g in range(NTILE):
                        nc.tensor.transpose(btp[0:1, g, :],
                                            nbeta[:, g:g + 1], ident[:])
                    bt = wpool.tile([1, NTILE, 128], BF16, tag="bt")
                    nc.vector.tensor_copy(bt[:], btp[:])
                    for g in range(NTILE):
                        k = NTILE * u + g
                        ps = ps_tiles[g]
                        # psum -= beta (x) h_n
                        nc.tensor.matmul(ps, lhsT=bt[0:1, g, :],
                                         rhs=hrow[0:1, u, :],
                                         start=False, stop=True,
                                         skip_group_check=True)
                        scol = wpool.tile([128, 1], F32, tag="scol")
                        nc.vector.tensor_reduce(
                            scol[:], ps, mybir.AxisListType.X, Alu.add,
                            apply_absolute_value=True)
                        # score = GAMMA - s/beta = s * (-rs) + GAMMA
                        nc.vector.tensor_scalar(
                            out=scu[:, k:k + 1],
                            in0=scol[:], scalar1=nrs[:, g:g + 1],
                            scalar2=GAMMA, op0=Alu.mult, op1=Alu.add)
                nc.sync.dma_start(out[:, ds(it * NC_IT, NC_IT)], scu[:])

    return nc


def make_in_maps(head, tail, relation, entity_emb, relation_emb, W_fc, b_fc):
    """Shard/pack FULL inputs into 8 per-core input maps.

    Returns (in_maps, nmaps, overflow) where nmaps[c][b, slot] = n (or -1
    for padding) maps device scores back, and overflow lists (b, n) pairs
    that did not fit (computed on host; probability ~0).
    """
    head = np.asarray(head).astype(np.int64).reshape(B_FULL)
    tail = np.asarray(tail).astype(np.int64)
    relation = np.asarray(relation).astype(np.int64).reshape(B_FULL)
    entity_emb = np.asarray(entity_emb, dtype=np.float32)
    relation_emb = np.asarray(relation_emb, dtype=np.float32)
    W_fc = np.asarray(W_fc, dtype=np.float32)
    b_fc = np.asarray(b_fc, dtype=np.float32).reshape(1, D)

    def quant_rows(x, nchunk, width):
        """Per-(partition, chunk) row int8 quant of a [128, nchunk*width]
        layout; returns (int8 data, [128, nchunk] f32 scales)."""
        r = x.reshape(128, nchunk, width)
        s = np.maximum(np.abs(r).max(axis=2), 1e-12) / 127.0   # [128, nchunk]
        q = np.round(r / s[:, :, None]).astype(np.int8)
        return (np.ascontiguousarray(q.reshape(128, nchunk * width)),
                np.ascontiguousarray(s.astype(np.float32)))

    # shared host-prepped operands (int8 + scales, matmul-ready layouts),
    # merged into one i8 tensor + one f32 scale tensor
    wt_f = np.ascontiguousarray(
        W_fc.T.reshape(4, 128, D).transpose(1, 0, 2).reshape(128, 4 * D))
    wt, wscl = quant_rows(wt_f, 4, D)
    hrows = entity_emb[head]                       # [B, D]
    ht_f = np.ascontiguousarray(
        hrows.T.reshape(2, 128, B_FULL).transpose(1, 0, 2).reshape(128, 2 * B_FULL))
    ht, hscl = quant_rows(ht_f, 2, B_FULL)
    rrows = relation_emb[relation]                 # [B, 2D]
    rt_f = np.ascontiguousarray(
        rrows.T.reshape(4, 128, B_FULL).transpose(1, 0, 2).reshape(128, 4 * B_FULL))
    rt, rscl = quant_rows(rt_f, 4, B_FULL)
    iops = np.ascontiguousarray(np.concatenate([wt, ht, rt], axis=1))
    scls = np.ascontiguousarray(np.concatenate(
        [wscl, hscl, rscl, b_fc.reshape(128, 2)], axis=1).astype(np.float32))
    shared_blob = np.concatenate([iops, scls.view(np.int8)], axis=1)
    # int8 per-row symmetric quantization of the entity table
    scl = np.maximum(np.abs(entity_emb).max(axis=1) / 127.0, 1e-12)  # [N]
    ent_q = np.round(entity_emb / scl[:, None]).astype(np.int8)

    owner = tail // SHARD
    local = (tail % SHARD).astype(np.int32)
    in_maps, nmaps, overflow = [], [], []
    for c in range(NCORES):
        mask = owner == c
        # compact the shard to referenced rows only (remapped indices)
        rows = np.unique(local[mask])
        if len(rows) > CROWS:
            # drop pairs referencing the rarest overflow rows -> host path
            drop_rows = set(rows[CROWS:].tolist())
            over_r = mask & np.isin(local, list(drop_rows))
            ob, on = np.nonzero(over_r)
            overflow.extend(zip(ob.tolist(), on.tolist()))
            mask = mask & ~over_r
            rows = rows[:CROWS]
        remap = np.zeros(SHARD, np.int32)
        remap[rows] = np.arange(len(rows), dtype=np.int32)
        ent_c = np.zeros((CROWS, D + 2), np.int8)
        ent_c[:len(rows), :D] = ent_q[c * SHARD + rows]
        ent_c[:len(rows), D:] = (
            scl[c * SHARD + rows].astype(np.float16).view(np.int8).reshape(-1, 2))

        pos = np.cumsum(mask, axis=1) - 1
        over = mask & (pos >= CAP)
        if over.any():
            ob, on = np.nonzero(over)
            overflow.extend(zip(ob.tolist(), on.tolist()))
            mask = mask & ~over
        idx_c = np.zeros((B_FULL, CAP), np.int32)
        nmap = np.full((B_FULL, CAP), -1, np.int64)
        bb, nn_ = np.nonzero(mask)
        p = pos[mask]
        idx_c[bb, p] = local[mask]
        nmap[bb, p] = nn_
        idx_cr = remap[idx_c]
        tidx_c = np.ascontiguousarray(
            idx_cr.reshape(NCOLS, 128).T.astype(np.uint16))
        in_maps.append({
            "ent": ent_c,
            "blob": np.ascontiguousarray(np.concatenate(
                [shared_blob, tidx_c.view(np.int8)], axis=1)),
        })
        nmaps.append(nmap)
    return in_maps, nmaps, overflow


def _host_scores(head, tail, relation, entity_emb, relation_emb, W_fc, b_fc,
                 pairs):
    """Exact numpy scores for a small list of (b, n) pairs (overflow path)."""
    b_idx = np.array([p[0] for p in pairs])
    n_idx = np.array([p[1] for p in pairs])
    h = entity_emb[head.reshape(-1)[b_idx]]
    t = entity_emb[tail[b_idx, n_idx]]
    r = relation_emb[relation.reshape(-1)[b_idx]]
    re_h, re_t = r[:, :D], r[:, D:]
    hc = np.concatenate([h, re_h], -1)
    tc = np.concatenate([t, re_t], -1)
    hf = hc @ W_fc.T + b_fc.reshape(-1)
    tf = tc @ W_fc.T + b_fc.reshape(-1)
    hn = hf / np.maximum(np.linalg.norm(hf, axis=-1, keepdims=True), 1e-12)
    tn = tf / np.maximum(np.linalg.norm(tf, axis=-1, keepdims=True), 1e-12)
    return b_idx, n_idx, GAMMA - np.abs(hn - tn).sum(-1)


def unshard_output(res, nmaps):
    """Scatter per-core packed scores back to the FULL [B, NEG] output."""
    score = np.zeros((B_FULL, NEG), dtype=np.float32)
    for c in range(NCORES):
        # out is [p, col]; col = b*NTILE + g, slot = g*128 + p
        s_c = np.ascontiguousarray(
            res[c]["out"].astype(np.float32).T).reshape(B_FULL, CAP)
        nmap = nmaps[c]
        vb, vs = np.nonzero(nmap >= 0)
        score[vb, nmap[vb, vs]] = s_c[vb, vs]
    return score


def kernel(head, tail, relation, entity_emb, relation_emb, W_fc, b_fc):
    nc = bacc.Bacc("TRN2", target_bir_lowering=False, debug=False)
    build_kernel(nc)
    nc.compile()
    in_maps, nmaps, overflow = make_in_maps(
        head, tail, relation, entity_emb, relation_emb, W_fc, b_fc)
    res = run_bass_kernel_spmd(nc, in_maps, core_ids=list(range(NCORES)))
    score = unshard_output(res.results, nmaps)
    if overflow:
        bi, ni, sv = _host_scores(
            np.asarray(head), np.asarray(tail), np.asarray(relation),
            np.asarray(entity_emb, np.float32),
            np.asarray(relation_emb, np.float32),
            np.asarray(W_fc, np.float32), np.asarray(b_fc, np.float32),
            overflow)
        score[bi, ni] = sv
    return score


# revision 54
# speedup vs baseline: 1.3737x; 1.1444x over previous
"""KGE scoring kernel for Trainium2 (8 NeuronCores, entity-table row-sharded).

score[b, n] = GAMMA - sum_d |h_n[b, d] - t_n[b, n, d]|
  h_n / t_n = L2-normalized Linear(concat(ent_emb[idx], rel_half))

The 200k x 256 entity table is row-sharded across the 8 cores, compacted
to referenced rows, and int8 per-row quantized (~5MB/core on the wire
instead of a 205MB f32 replica).  A tail pair (b, n) is computed on the
core owning row tail[b, n]; the host packs, per (core, b), the matching
n's into two 128-wide tiles (cap 256; Binomial(1024, 1/8) never exceeds
it, and a numpy fallback covers the impossible overflow) and scatters the
scalar scores back.  Head/relation rows and W are host-prepped (gather +
transpose + bf16); all FC compute, normalization and scoring stay on
device.

The main loop is a hardware For_i over batch rows (2 rows per iteration)
so the program stays ~150 instructions — full unrolling made every warm
call spend ~0.5s re-serializing and re-verifying a ~7k-instruction BIR.
Per-iteration operands (indices, scales, C_t/H_n rows) are staged with
small dynamically-sliced DMAs so compute APs stay static.

Per batch row b:
  t_fc = W1 @ t + C_t[b],  C_t = W2 @ re_t + b_fc  (per-b constant).
  After norm^2 (ACT Square+accum_out) and beta = ||t_fc||, a K=1 PE matmul
  accumulates -beta (x) h_n into the same PSUM, so
  score = GAMMA - (1/beta) * sum_d |psum|  (one DVE abs-add reduce per tile).
"""

import os
import sys

# Persistent XLA compilation cache: run_bass_via_pjrt builds a fresh jit
# closure per call, so without this every warm call re-runs the full
# client-side walrus/NEFF pipeline.  (No-op when the backend doesn't
# support serialized executables, but harmless.)
os.environ.setdefault("JAX_COMPILATION_CACHE_DIR", "/tmp/jax_comp_cache")
os.environ.setdefault("JAX_PERSISTENT_CACHE_MIN_COMPILE_TIME_SECS", "0")
os.environ.setdefault("JAX_PERSISTENT_CACHE_MIN_ENTRY_SIZE_BYTES", "0")

if "/opt/trn_rl_repo" not in sys.path:
    sys.path.insert(0, "/opt/trn_rl_repo")

import numpy as np

import concourse.bacc as bacc
import concourse.mybir as mybir
import concourse.tile as tile
from concourse.bass import IndirectOffsetOnAxis, ds
from concourse.bass_utils import run_bass_kernel_spmd
from concourse.masks import make_identity

GAMMA = 12.0
NENTITY = 200000
NREL = 500
D = 256          # hidden
B_FULL = 256     # total batch
NEG = 1024
NCORES = 8
SHARD = NENTITY // NCORES   # 25000 entity rows per core
# compacted shard capacity: only rows actually referenced are shipped
# (expected ~18.3k of 25k, observed max 18411; exact host fallback beyond)
CROWS = 18432
CAP = 256        # max pairs per (core, b); NTILE tiles of 128
NTILE = CAP // 128
NCOLS = B_FULL * NTILE      # 512 score columns per core
UNROLL = 2       # batch rows per hardware-loop iteration
BF16 = mybir.dt.bfloat16
F32 = mybir.dt.float32
I32 = mybir.dt.int32
I8 = mybir.dt.int8
U16 = mybir.dt.uint16
F16 = mybir.dt.float16
Square = mybir.ActivationFunctionType.Square
Alu = mybir.AluOpType


def build_kernel(nc):
    """Emit the SPMD per-core program."""
    # Inputs are merged into few tensors — the axon tunnel charges ~11ms
    # per input array on top of ~13ms/MB, so fewer/larger arrays win.
    # int8 per-row symmetric quantized compacted shard; cols D:D+2 of each
    # row carry its f16 dequant scale (bit-packed), so one gather brings
    # row + scale together
    ent = nc.dram_tensor("ent", [CROWS, D + 2], I8, kind="ExternalInput").ap()
    # everything else rides in ONE byte blob (bitcast views per region):
    #   bytes [0, 2560): int8 operands, pre-transposed on host:
    #     dequant([:, 256j:...])[p, d]       = W_fc[d, 128*j + p]        j<4
    #     dequant([:, 1024+256j:...])[p, b]  = ent_emb[head[b], 128j+p]  j<2
    #     dequant([:, 1536+256j:...])[p, b]  = rel_emb[relation[b], 128j+p]
    #   bytes [2560, 2608): f32 dequant scales w[0:4] | h[4:6] | r[6:10]
    #     and the f32 bias as [128, 2] in scale cols 10:12 (relayout to
    #     [1, 256] on device via a DRAM round trip)
    #   bytes [2608, 3632): u16 packed local tail rows
    #     (col r = b*NTILE + g, row p -> slot g*128 + p)
    IOPS_B, SCLS_B, TIDX_B = 0, 10 * D, 10 * D + 48
    blob = nc.dram_tensor("blob", [128, TIDX_B + 2 * NCOLS], I8,
                          kind="ExternalInput").ap()
    # scores in [p, col] layout; host transposes
    out = nc.dram_tensor("out", [128, NCOLS], F16, kind="ExternalOutput").ap()

    with tile.TileContext(nc) as tc:
        with (
            tc.tile_pool(name="const", bufs=1) as cpool,
            tc.tile_pool(name="stage", bufs=2) as spool,
            tc.tile_pool(name="gath", bufs=2) as gpool,
            tc.tile_pool(name="tt", bufs=2) as ttpool,
            tc.tile_pool(name="work", bufs=2) as wpool,
            tc.tile_pool(name="dram", bufs=1, space="DRAM") as dpool,
            tc.tile_pool(name="pstt", bufs=2, space="PSUM") as ps_tt,
            tc.tile_pool(name="psbt", bufs=1, space="PSUM") as ps_bt,
            tc.tile_pool(name="psmain", bufs=2, space="PSUM") as psmain,
        ):
            # ---- constants ----
            ident = cpool.tile([128, 128], BF16)
            make_identity(nc, ident[:])
            ones_row = cpool.tile([1, 128], BF16)
            nc.vector.memset(ones_row[:], 1.0)

            # load int8 operands + dequant to bf16 (per-partition-row scales)
            iq = cpool.tile([128, 10, D], I8, tag="iq")
            nc.sync.dma_start(iq[:], blob[:, IOPS_B:IOPS_B + 10 * D])
            scl_sb = cpool.tile([128, 12], F32, tag="scl")
            nc.sync.dma_start(scl_sb[:],
                              blob[:, SCLS_B:SCLS_B + 48].bitcast(F32))
            wt = cpool.tile([128, 4, D], BF16, tag="wt")
            for j in range(4):
                nc.vector.tensor_scalar_mul(wt[:, j, :], iq[:, j, :],
                                            scl_sb[:, j:j + 1])
            ht = cpool.tile([128, 2, B_FULL], BF16, tag="ht")
            for j in range(2):
                nc.vector.tensor_scalar_mul(ht[:, j, :], iq[:, 4 + j, :],
                                            scl_sb[:, 4 + j:5 + j])
            rt = cpool.tile([128, 4, B_FULL], BF16, tag="rt")
            for j in range(4):
                nc.vector.tensor_scalar_mul(rt[:, j, :], iq[:, 6 + j, :],
                                            scl_sb[:, 6 + j:7 + j])
            # bias rides in scls cols 10:12; relayout [128, 2] -> [1, 256]
            bd = dpool.tile([128, 2], F32, tag="bd")
            nc.sync.dma_start(bd[:], scl_sb[:, 10:12])
            b_f32 = cpool.tile([1, D], F32, tag="bf32")
            nc.sync.dma_start(b_f32[:], bd[:, :])
            b_bf = cpool.tile([1, D], BF16, tag="bias")
            nc.vector.tensor_copy(b_bf[:], b_f32[:])

            # ---- per-b constants for ALL 256 b, in two chunks of 128 ----
            # C_t[b,:] = W2 @ re_t[b] + b_fc ; Hn[b,:] = normalize(FC(head))
            ctd = dpool.tile([B_FULL, D], BF16, tag="ctd")
            hnd = dpool.tile([B_FULL, D], BF16, tag="hnd")
            for ch in range(2):
                bs = 128 * ch
                ct_ps = ps_tt.tile([128, D], F32, tag="ttp")
                nc.tensor.matmul(ct_ps[:], lhsT=ones_row[:], rhs=b_bf[:],
                                 start=True, stop=False)
                nc.tensor.matmul(ct_ps[:], lhsT=rt[:, 2, bs:bs + 128],
                                 rhs=wt[:, 2, :], start=False, stop=False)
                nc.tensor.matmul(ct_ps[:], lhsT=rt[:, 3, bs:bs + 128],
                                 rhs=wt[:, 3, :], start=False, stop=True)
                ct = wpool.tile([128, D], BF16, tag="ct")
                nc.scalar.copy(ct[:], ct_ps[:])
                nc.sync.dma_start(ctd[bs:bs + 128, :], ct[:])

                hf_ps = ps_tt.tile([128, D], F32, tag="ttp")
                nc.tensor.matmul(hf_ps[:], lhsT=ones_row[:], rhs=b_bf[:],
                                 start=True, stop=False)
                nc.tensor.matmul(hf_ps[:], lhsT=ht[:, 0, bs:bs + 128],
                                 rhs=wt[:, 0, :], start=False, stop=False)
                nc.tensor.matmul(hf_ps[:], lhsT=ht[:, 1, bs:bs + 128],
                                 rhs=wt[:, 1, :], start=False, stop=False)
                nc.tensor.matmul(hf_ps[:], lhsT=rt[:, 0, bs:bs + 128],
                                 rhs=wt[:, 2, :], start=False, stop=False)
                nc.tensor.matmul(hf_ps[:], lhsT=rt[:, 1, bs:bs + 128],
                                 rhs=wt[:, 3, :], start=False, stop=True)
                h_sq = wpool.tile([128, D], BF16, tag="hsq")
                h_nn = wpool.tile([128, 1], F32, tag="hnn")
                nc.scalar.activation(h_sq[:], hf_ps[:], Square, accum_out=h_nn[:])
                h_beta = wpool.tile([128, 1], F32, tag="hbeta")
                nc.scalar.sqrt(h_beta[:], h_nn[:])
                h_rs = wpool.tile([128, 1], F32, tag="hrs")
                nc.vector.reciprocal_approx_fast(h_rs[:], h_beta[:])
                hn = wpool.tile([128, D], BF16, tag="hn")
                nc.vector.tensor_scalar_mul(hn[:], hf_ps[:], h_rs[:, :1])
                nc.sync.dma_start(hnd[bs:bs + 128, :], hn[:])

            # ---- main hardware loop: UNROLL batch rows per iteration ----
            NC_IT = NTILE * UNROLL      # score cols per iteration
            with tc.For_i(0, B_FULL // UNROLL, 1) as it:
                # stage this iteration's operands (dynamic DRAM slices)
                tix = spool.tile([128, NC_IT], U16, tag="tix")
                nc.sync.dma_start(
                    tix[:],
                    blob[:, ds(TIDX_B + it * 2 * NC_IT, 2 * NC_IT)].bitcast(U16))
                tii = spool.tile([128, NC_IT], I32, tag="tii")
                nc.vector.tensor_copy(tii[:], tix[:])
                crow = spool.tile([1, UNROLL, D], BF16, tag="crow")
                nc.sync.dma_start(crow[:], ctd[ds(it * UNROLL, UNROLL), :])
                hrow = spool.tile([1, UNROLL, D], BF16, tag="hrow")
                nc.sync.dma_start(hrow[:], hnd[ds(it * UNROLL, UNROLL), :])
                scu = spool.tile([128, NC_IT], F16, tag="scu")

                for u in range(UNROLL):
                    # gather 2x128 packed tail rows (one DMA per 128-row
                    # tile: single-column offset APs only — multi-column
                    # offsets misbehave on HW SWDGE)
                    gq = gpool.tile([128, NTILE, D + 2], I8, tag="gq")
                    for g in range(NTILE):
                        k = NTILE * u + g
                        nc.gpsimd.indirect_dma_start(
                            out=gq[:, g, :], out_offset=None, in_=ent[:],
                            in_offset=IndirectOffsetOnAxis(
                                ap=tii[:, k:k + 1], axis=0))
                    # dequant: t = q * s_row; the row's f16 scale rides in
                    # its last 2 bytes (widen to f32 for the scalar AP)
                    gsc = wpool.tile([128, NTILE], F32, tag="gsc")
                    for g in range(NTILE):
                        nc.vector.tensor_copy(
                            gsc[:, g:g + 1], gq[:, g, D:D + 2].bitcast(F16))
                    gt = gpool.tile([128, NTILE, D], BF16, tag="gt")
                    for g in range(NTILE):
                        nc.vector.tensor_scalar_mul(
                            gt[:, g, :], gq[:, g, 0:D], gsc[:, g:g + 1])
                    nn2 = wpool.tile([128, NTILE], F32, tag="nn2")
                    ps_tiles = [psmain.tile([128, D], F32, tag="psm",
                                            name=f"psm_{u}_{i}")[:]
                                for i in range(NTILE)]
                    for g in range(NTILE):
                        # transpose tile -> TT [128, 2, 128] (k-chunk, rows)
                        ttp = ps_tt.tile([128, 2, 128], BF16, tag="ttp")
                        nc.tensor.transpose(ttp[:, 0, :], gt[:, g, 0:128],
                                            ident[:])
                        nc.tensor.transpose(ttp[:, 1, :], gt[:, g, 128:256],
                                            ident[:])
                        tt = ttpool.tile([128, 2, 128], BF16, tag="tt")
                        nc.scalar.copy(tt[:, 0, :], ttp[:, 0, :])
                        nc.vector.tensor_copy(tt[:, 1, :], ttp[:, 1, :])
                        # psum = C_t[b] + W1 @ t
                        ps = ps_tiles[g]
                        nc.tensor.matmul(ps, lhsT=ones_row[:],
                                         rhs=crow[0:1, u, :],
                                         start=True, stop=False)
                        nc.tensor.matmul(ps, lhsT=tt[:, 0, :],
                                         rhs=wt[:, 0, :],
                                         start=False, stop=False)
                        nc.tensor.matmul(ps, lhsT=tt[:, 1, :],
                                         rhs=wt[:, 1, :],
                                         start=False, stop=True)
                        # norm^2 -> nn2 col g
                        sq = wpool.tile([128, D], BF16, tag="sq")
                        nc.scalar.activation(sq[:], ps, Square,
                                             accum_out=nn2[:, g:g + 1])
                    # beta = sqrt(nn); negated row form for the K=1 correction
                    beta = wpool.tile([128, NTILE], F32, tag="beta")
                    nc.scalar.sqrt(beta[:], nn2[:])
                    nbeta = wpool.tile([128, NTILE], BF16, tag="nbeta")
                    nc.vector.tensor_scalar_mul(nbeta[:], beta[:], -1.0)
                    rs = wpool.tile([128, NTILE], F32, tag="rs")
                    nc.vector.reciprocal_approx_fast(rs[:], beta[:])
                    nrs = wpool.tile([128, NTILE], F32, tag="nrs")
                    nc.vector.tensor_scalar_mul(nrs[:], rs[:], -1.0)
                    btp = ps_bt.tile([1, NTILE, 128], BF16, tag="btp")
                    for g in range(NTILE):
                        nc.tensor.transpose(btp[0:1, g, :],
                                            nbeta[:, g:g + 1], ident[:])
                    bt = wpool.tile([1, NTILE, 128], BF16, tag="bt")
                    nc.vector.tensor_copy(bt[:], btp[:])
                    for g in range(NTILE):
                        k = NTILE * u + g
                        ps = ps_tiles[g]
                        # psum -= beta (x) h_n
                        nc.tensor.matmul(ps, lhsT=bt[0:1, g, :],
                                         rhs=hrow[0:1, u, :],
                                         start=False, stop=True,
                                         skip_group_check=True)
                        scol = wpool.tile([128, 1], F32, tag="scol")
                        nc.vector.tensor_reduce(
                            scol[:], ps, mybir.AxisListType.X, Alu.add,
                            apply_absolute_value=True)
                        # score = GAMMA - s/beta = s * (-rs) + GAMMA
                        nc.vector.tensor_scalar(
                            out=scu[:, k:k + 1],
                            in0=scol[:], scalar1=nrs[:, g:g + 1],
                            scalar2=GAMMA, op0=Alu.mult, op1=Alu.add)
                nc.sync.dma_start(out[:, ds(it * NC_IT, NC_IT)], scu[:])

    return nc


def make_in_maps(head, tail, relation, entity_emb, relation_emb, W_fc, b_fc):
    """Shard/pack FULL inputs into 8 per-core input maps.

    Returns (in_maps, nmaps, overflow) where nmaps[c][b, slot] = n (or -1
    for padding) maps device scores back, and overflow lists (b, n) pairs
    that did not fit (computed on host; probability ~0).
    """
    head = np.asarray(head).astype(np.int64).reshape(B_FULL)
    tail = np.asarray(tail).astype(np.int64)
    relation = np.asarray(relation).astype(np.int64).reshape(B_FULL)
    entity_emb = np.asarray(entity_emb, dtype=np.float32)
    relation_emb = np.asarray(relation_emb, dtype=np.float32)
    W_fc = np.asarray(W_fc, dtype=np.float32)
    b_fc = np.asarray(b_fc, dtype=np.float32).reshape(1, D)

    def quant_rows(x, nchunk, width):
        """Per-(partition, chunk) row int8 quant of a [128, nchunk*width]
        layout; returns (int8 data, [128, nchunk] f32 scales)."""
        r = x.reshape(128, nchunk, width)
        s = np.maximum(np.abs(r).max(axis=2), 1e-12) / 127.0   # [128, nchunk]
        q = np.round(r / s[:, :, None]).astype(np.int8)
        return (np.ascontiguousarray(q.reshape(128, nchunk * width)),
                np.ascontiguousarray(s.astype(np.float32)))

    # shared host-prepped operands (int8 + scales, matmul-ready layouts),
    # merged into one i8 tensor + one f32 scale tensor
    wt_f = np.ascontiguousarray(
        W_fc.T.reshape(4, 128, D).transpose(1, 0, 2).reshape(128, 4 * D))
    wt, wscl = quant_rows(wt_f, 4, D)
    hrows = entity_emb[head]                       # [B, D]
    ht_f = np.ascontiguousarray(
        hrows.T.reshape(2, 128, B_FULL).transpose(1, 0, 2).reshape(128, 2 * B_FULL))
    ht, hscl = quant_rows(ht_f, 2, B_FULL)
    rrows = relation_emb[relation]                 # [B, 2D]
    rt_f = np.ascontiguousarray(
        rrows.T.reshape(4, 128, B_FULL).transpose(1, 0, 2).reshape(128, 4 * B_FULL))
    rt, rscl = quant_rows(rt_f, 4, B_FULL)
    iops = np.ascontiguousarray(np.concatenate([wt, ht, rt], axis=1))
    scls = np.ascontiguousarray(np.concatenate(
        [wscl, hscl, rscl, b_fc.reshape(128, 2)], axis=1).astype(np.float32))
    shared_blob = np.concatenate([iops, scls.view(np.int8)], axis=1)
    # int8 per-row symmetric quantization of the entity table
    scl = np.maximum(np.abs(entity_emb).max(axis=1) / 127.0, 1e-12)  # [N]
    ent_q = np.round(entity_emb / scl[:, None]).astype(np.int8)

    owner = tail // SHARD
    local = (tail % SHARD).astype(np.int32)
    in_maps, nmaps, overflow = [], [], []
    for c in range(NCORES):
        mask = owner == c
        # compact the shard to referenced rows only (remapped indices)
        rows = np.unique(local[mask])
        if len(rows) > CROWS:
            # drop pairs referencing the rarest overflow rows -> host path
            drop_rows = set(rows[CROWS:].tolist())
            over_r = mask & np.isin(local, list(drop_rows))
            ob, on = np.nonzero(over_r)
            overflow.extend(zip(ob.tolist(), on.tolist()))
            mask = mask & ~over_r
            rows = rows[:CROWS]
        remap = np.zeros(SHARD, np.int32)
        remap[rows] = np.arange(len(rows), dtype=np.int32)
        ent_c = np.zeros((CROWS, D + 2), np.int8)
        ent_c[:len(rows), :D] = ent_q[c * SHARD + rows]
        ent_c[:len(rows), D:] = (
            scl[c * SHARD + rows].astype(np.float16).view(np.int8).reshape(-1, 2))

        pos = np.cumsum(mask, axis=1) - 1
        over = mask & (pos >= CAP)
        if over.any():
            ob, on = np.nonzero(over)
            overflow.extend(zip(ob.tolist(), on.tolist()))
            mask = mask & ~over
        idx_c = np.zeros((B_FULL, CAP), np.int32)
        nmap = np.full((B_FULL, CAP), -1, np.int64)
        bb, nn_ = np.nonzero(mask)
        p = pos[mask]
        idx_c[bb, p] = local[mask]
        nmap[bb, p] = nn_
        idx_cr = remap[idx_c]
        tidx_c = np.ascontiguousarray(
            idx_cr.reshape(NCOLS, 128).T.astype(np.uint16))
        in_maps.append({
            "ent": ent_c,
            "blob": np.ascontiguousarray(np.concatenate(
                [shared_blob, tidx_c.view(np.int8)], axis=1)),
        })
        nmaps.append(nmap)
    return in_maps, nmaps, overflow


def _host_scores(head, tail, relation, entity_emb, relation_emb, W_fc, b_fc,
                 pairs):
    """Exact numpy scores for a small list of (b, n) pairs (overflow path)."""
    b_idx = np.array([p[0] for p in pairs])
    n_idx = np.array([p[1] for p in pairs])
    h = entity_emb[head.reshape(-1)[b_idx]]
    t = entity_emb[tail[b_idx, n_idx]]
    r = relation_emb[relation.reshape(-1)[b_idx]]
    re_h, re_t = r[:, :D], r[:, D:]
    hc = np.concatenate([h, re_h], -1)
    tc = np.concatenate([t, re_t], -1)
    hf = hc @ W_fc.T + b_fc.reshape(-1)
    tf = tc @ W_fc.T + b_fc.reshape(-1)
    hn = hf / np.maximum(np.linalg.norm(hf, axis=-1, keepdims=True), 1e-12)
    tn = tf / np.maximum(np.linalg.norm(tf, axis=-1, keepdims=True), 1e-12)
    return b_idx, n_idx, GAMMA - np.abs(hn - tn).sum(-1)


def unshard_output(res, nmaps):
    """Scatter per-core packed scores back to the FULL [B, NEG] output."""
    score = np.zeros((B_FULL, NEG), dtype=np.float32)
    for c in range(NCORES):
        # out is [p, col]; col = b*NTILE + g, slot = g*128 + p
        s_c = np.ascontiguousarray(
            res[c]["out"].astype(np.float32).T).reshape(B_FULL, CAP)
        nmap = nmaps[c]
        vb, vs = np.nonzero(nmap >= 0)
        score[vb, nmap[vb, vs]] = s_c[vb, vs]
    return score


def kernel(head, tail, relation, entity_emb, relation_emb, W_fc, b_fc):
    nc = bacc.Bacc("TRN2", target_bir_lowering=False, debug=False)
    build_kernel(nc)
    nc.compile()
    in_maps, nmaps, overflow = make_in_maps(
        head, tail, relation, entity_emb, relation_emb, W_fc, b_fc)
    res = run_bass_kernel_spmd(nc, in_maps, core_ids=list(range(NCORES)))
    score = unshard_output(res.results, nmaps)
    if overflow:
        bi, ni, sv = _host_scores(
            np.asarray(head), np.asarray(tail), np.asarray(relation),
            np.asarray(entity_emb, np.float32),
            np.asarray(relation_emb, np.float32),
            np.asarray(W_fc, np.float32), np.asarray(b_fc, np.float32),
            overflow)
        score[bi, ni] = sv
    return score
